# revision 14
# baseline (speedup 1.0000x reference)
"""Bahdanau-attention kernel for 8 Trainium2 NeuronCores (SPMD, batch-sharded).

Algorithm: scores[t,s] = sum_h v_h * tanh(D[h,t] + E[h,s]) via a free-frequency
sine expansion  tanh(x) ~= sum_k b_k sin(w_k x)  (F=4, Gaussian-weighted fit),
factored through the angle-addition formula into 2F PSUM-accumulating matmuls
over sin/cos features of uD = W2^T dec^T and uE = W1^T enc^T (each computed
once as a bf16 matmul; PSUM layout [uE(512)|uD(256)]).

Per-frequency features (wide [128,768] e||d instructions):
  k=0:  |w0*u| <= ~1.6 fits the Sin LUT domain (~+-3.55): fS = Sin(w0*u),
        fC = Sin(w0*u + pi/2) directly from PSUM. No range reduction.
  k=1:  fS = Sin(w1*u) (rare |arg|>3.55 values clamp; error negligible),
        fC = Sin(-w1*|u| + pi/2) via one DVE abs.
  k=2,3: magic-constant range reduction on DVE (f32): v = u*s_k;
        i = (v+M)-M; a = v-i; b = |a|; then Sin(2pi*a), Sin(-2pi*b + pi/2).
v*b_k folds into the decoder features (bf16 per-partition tensor_scalar).
Encoder padding mask enters PSUM as a -1e30 seed via a K=1 rank-1 bf16 matmul;
softmax runs without max-shift (bf16 exp + f32 accum); the decoder mask folds
into the 1/sum scale. Inputs arrive via 4 packed DMAs; transposes/casts are
host-side layout prep.
"""
import os
import sys

import numpy as np

if "/opt/trn_rl_repo" not in sys.path:
    sys.path.insert(0, "/opt/trn_rl_repo")

S, T, B, H = 512, 256, 8, 128
F = 4
OMEGA = np.array(
    [0.28378837870584145, 0.8460440611349613,
     1.5065032153851483, 2.6337314948021557], dtype=np.float64
)
BK = np.array(
    [1.2424371140304091, 0.31247076876828433,
     0.16201975442700692, 0.045413278208112536], dtype=np.float64
)
TWO_PI = float(2.0 * np.pi)
PI = float(np.pi)
HALF_PI = float(0.5 * np.pi)
NEG_BIG = -1.0e30

_CACHE = {}
LAST_EXEC_NS = None


def _try_install_trace_hook():
    """Best-effort NTFF profile hook for axon (used only when tracing)."""
    try:
        import contextlib
        import ctypes
        import types

        if "antenv.axon_hooks" in sys.modules:
            return
        lib = ctypes.CDLL("/opt/axon/libaxon_pjrt.so")
        if not hasattr(lib, "axon_start_nrt_profile"):
            return
        lib.axon_start_nrt_profile.argtypes = [
            ctypes.POINTER(ctypes.c_int64),
            ctypes.c_size_t,
        ]
        lib.axon_start_nrt_profile.restype = ctypes.c_int64
        lib.axon_stop_nrt_profile.argtypes = [ctypes.c_char_p]
        lib.axon_stop_nrt_profile.restype = ctypes.c_int64

        @contextlib.contextmanager
        def _hook(output_dir, device_ids):
            import jax

            jax.devices()
            if device_ids:
                ids = (ctypes.c_int64 * len(device_ids))(*device_ids)
                rc = lib.axon_start_nrt_profile(ids, len(device_ids))
            else:
                rc = lib.axon_start_nrt_profile(None, 0)
            if rc != 0:
                raise RuntimeError(f"axon_start_nrt_profile rc={rc}")
            try:
                yield
            finally:
                n = lib.axon_stop_nrt_profile(str(output_dir).encode())
                if n < 0:
                    raise RuntimeError(f"axon_stop_nrt_profile rc={n}")

        mod = types.ModuleType("antenv.axon_hooks")
        _h = _hook

        def set_axon_ntff_profile_hook(h):
            pass

        def get_axon_ntff_profile_hook():
            return _h

        mod.set_axon_ntff_profile_hook = set_axon_ntff_profile_hook
        mod.get_axon_ntff_profile_hook = get_axon_ntff_profile_hook
        sys.modules["antenv.axon_hooks"] = mod
        import antenv

        antenv.axon_hooks = mod
    except Exception:
        pass


def _build():
    if "nc" in _CACHE:
        return _CACHE["nc"]
    import concourse.bacc as bacc
    import concourse.tile as tile
    import concourse.mybir as mybir

    F32 = mybir.dt.float32
    BF16 = mybir.dt.bfloat16
    FP16 = mybir.dt.float16
    AF = mybir.ActivationFunctionType
    AL = mybir.AluOpType

    SCAL = [float(w / (2.0 * np.pi)) for w in OMEGA]

    nc = bacc.Bacc("TRN2", target_bir_lowering=False, debug=False, num_devices=8)

    p1_d = nc.dram_tensor("p1", [H, H + T], BF16, kind="ExternalInput")
    p2_d = nc.dram_tensor("p2", [H, H + S], BF16, kind="ExternalInput")
    p3_d = nc.dram_tensor("p3", [H, F + 2], F32, kind="ExternalInput")
    em_d = nc.dram_tensor("encmask", [1, S], BF16, kind="ExternalInput")
    out_d = nc.dram_tensor("out", [T, S], F32, kind="ExternalOutput")

    with tile.TileContext(nc) as tc:
        with (
            tc.tile_pool(name="cst", bufs=1) as cst,
            tc.tile_pool(name="wrk", bufs=1) as wrk,
            tc.tile_pool(name="ps", bufs=1, space="PSUM") as psp,
        ):
            with nc.named_scope("dma_in"):
                p1_sb = cst.tile([H, H + T], BF16)
                nc.sync.dma_start(p1_sb[:], p1_d[:])
                p2_sb = cst.tile([H, H + S], BF16)
                nc.sync.dma_start(p2_sb[:], p2_d[:])
                p3_sb = cst.tile([H, F + 2], F32)
                nc.sync.dma_start(p3_sb[:], p3_d[:])
                em_sb = cst.tile([1, S], BF16)
                nc.sync.dma_start(em_sb[:], em_d[:])

            ones_sb = cst.tile([1, H], BF16)
            nc.gpsimd.memset(ones_sb[:], 1.0)
            hp_sb = cst.tile([128, 1], F32)
            nc.gpsimd.memset(hp_sb[:], HALF_PI)

            # ---- u = [uE(512) | uD(256)] in PSUM, bank-aligned ----
            u_ps = psp.tile([128, 768], F32, tag="ups")
            uE = u_ps[:, 0:S]
            uD = u_ps[:, S:S + T]
            uA = u_ps[:, 0:S + T]
            with nc.named_scope("u_mm"):
                nc.tensor.matmul(
                    uD, p1_sb[:, 0:H], p1_sb[:, H:], start=True, stop=True)
                nc.tensor.matmul(
                    uE, p2_sb[:, 0:H], p2_sb[:, H:], start=True, stop=True)

            # ---- score PSUM seeded with -1e30 encoder mask ----
            sc = []
            for tb in range(2):
                sc_tile = psp.tile([128, S], F32, tag=f"sc{tb}")
                sc.append(sc_tile)
                with nc.named_scope(f"mask_{tb}"):
                    nc.tensor.matmul(
                        sc_tile[:], ones_sb[:], em_sb[:],
                        start=True, stop=False, skip_group_check=True,
                    )

            W = S + T  # 768: all feature tiles are [e(512) | d(256)]
            M32 = float(1.5 * 2**23)

            def folds(k, fS, fC):
                """v*b_k folds into d-side features -> new [128,T] bf16 tiles."""
                with nc.named_scope(f"vfold_{k}"):
                    fSdv = wrk.tile([128, T], BF16, name=f"fSdv{k}")
                    nc.vector.tensor_scalar_mul(
                        fSdv[:], fS[:, S:], p3_sb[:, k:k + 1])
                    fCdv = wrk.tile([128, T], BF16, name=f"fCdv{k}")
                    nc.vector.tensor_scalar_mul(
                        fCdv[:], fC[:, S:], p3_sb[:, k:k + 1])
                return fSdv, fCdv

            def scores(k, fSdv, fCdv, fS, fC):
                with nc.named_scope(f"scores_{k}"):
                    last = k == F - 1
                    for tb in range(2):
                        dsl = slice(tb * 128, (tb + 1) * 128)
                        nc.tensor.matmul(
                            sc[tb][:], fSdv[:, dsl], fC[:, 0:S],
                            start=False, stop=False, skip_group_check=True,
                        )
                        nc.tensor.matmul(
                            sc[tb][:], fCdv[:, dsl], fS[:, 0:S],
                            start=False, stop=last, skip_group_check=True,
                        )

            # Preload the Exp table set into the second table slot while
            # ACT is otherwise idle, so the softmax needs no mid-stream load.
            scr = wrk.tile([128, 1], F32, name="scr")
            nc.scalar.activation(scr[:], hp_sb[:], AF.Exp)

            # ---- features, interleaved for pipelining ----
            # DVE order: negu, v2, i2, folds0, v3, i3, folds1, b2, b3,
            #            folds2, folds3, softmax.  GPSIMD: absu, a2, a3.
            # ACT: fS0,fC0,fS1,fC1,fS2,fC2,fS3,fC3,exp0,exp1.
            with nc.named_scope("sin_0"):
                fS0 = wrk.tile([128, W], BF16, name="fS0")
                nc.scalar.activation(fS0[:], uA, AF.Sin, scale=float(OMEGA[0]))
                fC0 = wrk.tile([128, W], BF16, name="fC0")
                nc.scalar.activation(
                    fC0[:], uA, AF.Sin, bias=hp_sb[:], scale=float(OMEGA[0]))

            negu = wrk.tile([128, W], F32, name="negu")
            nc.vector.tensor_scalar_mul(negu[:], uA, -1.0)
            absu = wrk.tile([128, W], F32, name="absu")
            nc.vector.tensor_tensor(absu[:], uA, negu[:], AL.max)

            with nc.named_scope("sin_1"):
                fS1 = wrk.tile([128, W], BF16, name="fS1")
                nc.scalar.activation(fS1[:], uA, AF.Sin, scale=float(OMEGA[1]))
                fC1 = wrk.tile([128, W], BF16, name="fC1")
                nc.scalar.activation(
                    fC1[:], absu[:], AF.Sin, bias=hp_sb[:],
                    scale=float(-OMEGA[1]))

            chain = {}
            for k in (2, 3):
                with nc.named_scope(f"red_{k}"):
                    vv = wrk.tile([128, W], F32, name=f"v{k}")
                    nc.vector.tensor_scalar_mul(vv[:], uA, SCAL[k])
                    ii = wrk.tile([128, W], F32, name=f"i{k}")
                    nc.vector.tensor_scalar(
                        ii[:], vv[:], M32, M32, AL.add, AL.subtract)
                    chain[k] = (vv, ii)
                if k == 2:
                    fS0v, fC0v = folds(0, fS0, fC0)
                    scores(0, fS0v, fC0v, fS0, fC0)
            fS1v, fC1v = folds(1, fS1, fC1)
            scores(1, fS1v, fC1v, fS1, fC1)

            feats = {}
            for k in (2, 3):
                vv, ii = chain[k]
                with nc.named_scope(f"red2_{k}"):
                    aa = wrk.tile([128, W], F32, name=f"a{k}")
                    nc.gpsimd.tensor_tensor(aa[:], vv[:], ii[:], AL.subtract)
                    bb = wrk.tile([128, W], F32, name=f"b{k}")
                    nc.vector.scalar_tensor_tensor(
                        bb[:], aa[:], -1.0, aa[:], AL.mult, AL.max)
                with nc.named_scope(f"sin_{k}"):
                    fSk = wrk.tile([128, W], BF16, name=f"fS{k}")
                    nc.scalar.activation(fSk[:], aa[:], AF.Sin, scale=TWO_PI)
                    fCk = wrk.tile([128, W], BF16, name=f"fC{k}")
                    nc.scalar.activation(
                        fCk[:], bb[:], AF.Sin, bias=hp_sb[:], scale=-TWO_PI)
                feats[k] = (fSk, fCk)
            for k in (2, 3):
                fSk, fCk = feats[k]
                fSkv, fCkv = folds(k, fSk, fCk)
                scores(k, fSkv, fCkv, fSk, fCk)

            # ---- softmax + decoder-mask scale + store ----
            for tb in range(2):
                with nc.named_scope(f"softmax_{tb}"):
                    ex = wrk.tile([128, S], BF16, name=f"ex{tb}")
                    rs = wrk.tile([128, 1], F32, name=f"rs{tb}")
                    nc.scalar.activation(ex[:], sc[tb][:], AF.Exp, accum_out=rs[:])
                    ri = wrk.tile([128, 1], F32, name=f"ri{tb}")
                    nc.vector.reciprocal(ri[:], rs[:])
                    fac = wrk.tile([128, 1], F32, name=f"fac{tb}")
                    nc.vector.tensor_tensor(
                        fac[:], ri[:], p3_sb[:, F + tb:F + tb + 1],
                        mybir.AluOpType.mult)
                    ot = wrk.tile([128, S], F32, name=f"ot{tb}")
                    nc.vector.tensor_scalar_mul(ot[:], ex[:], fac[:])
                    nc.sync.dma_start(out_d[tb * 128:(tb + 1) * 128, :], ot[:])

    nc.compile()
    _CACHE["nc"] = nc
    return nc


def kernel(encoder_output, decoder_output, W1, W2, v, enc_lens, dec_lens):
    global LAST_EXEC_NS
    from concourse.bass_utils import run_bass_kernel_spmd
    import ml_dtypes

    BF = ml_dtypes.bfloat16
    enc = np.asarray(encoder_output, dtype=np.float32)
    dec = np.asarray(decoder_output, dtype=np.float32)
    W1 = np.asarray(W1, dtype=np.float32)
    W2 = np.asarray(W2, dtype=np.float32)
    v = np.asarray(v, dtype=np.float32)
    enc_lens = np.asarray(enc_lens)
    dec_lens = np.asarray(dec_lens)

    vb = (v[:, None].astype(np.float64) * BK[None, :]).astype(np.float32)  # (H,F)

    in_maps = []
    for b in range(B):
        p1 = np.ascontiguousarray(
            np.concatenate([W2, dec[:, b, :].T], axis=1)).astype(BF)
        p2 = np.ascontiguousarray(
            np.concatenate([W1, enc[:, b, :].T], axis=1)).astype(BF)
        dm = (np.arange(T) < int(dec_lens[b])).astype(np.float32)
        p3 = np.ascontiguousarray(
            np.concatenate([vb, dm.reshape(H, 2, order="F")], axis=1))
        em = np.where(
            np.arange(S)[None, :] < int(enc_lens[b]), 0.0, NEG_BIG
        ).astype(BF)
        in_maps.append({"p1": p1, "p2": p2, "p3": p3, "encmask": em})

    trace = os.environ.get("KERNEL_TRACE", "0") == "1"
    if trace:
        _try_install_trace_hook()
    nc = _build()
    ncores = int(os.environ.get("KERNEL_CORES", str(B)))
    res = run_bass_kernel_spmd(nc, in_maps[:ncores], core_ids=list(range(ncores)), trace=trace)
    if trace:
        LAST_EXEC_NS = res.exec_time_ns
        _CACHE["last_res"] = res

    out = np.zeros((T, B, S), dtype=np.float32)
    for b in range(ncores):
        out[:, b, :] = res.results[b]["out"]
    return out


# revision 16
# speedup vs baseline: 1.1518x; 1.1518x over previous
"""Bahdanau-attention kernel for 8 Trainium2 NeuronCores (SPMD, batch-sharded).

Algorithm: scores[t,s] = sum_h v_h * tanh(D[h,t] + E[h,s]) via a free-frequency
sine expansion  tanh(x) ~= sum_k b_k sin(w_k x)  (F=4, Gaussian-weighted fit),
factored through the angle-addition formula into 2F PSUM-accumulating matmuls
over sin/cos features of uD = W2^T dec^T and uE = W1^T enc^T (each computed
once as a bf16 matmul; PSUM layout [uE(512)|uD(256)]).

Per-frequency features (wide [128,768] e||d instructions):
  k=0:  |w0*u| <= ~1.6 fits the Sin LUT domain (~+-3.55): fS = Sin(w0*u),
        fC = Sin(w0*u + pi/2) directly from PSUM. No range reduction.
  k=1:  fS = Sin(w1*u) (rare |arg|>3.55 values clamp; error negligible),
        fC = Sin(-w1*|u| + pi/2) via one DVE abs.
  k=2,3: magic-constant range reduction on DVE (f32): v = u*s_k;
        i = (v+M)-M; a = v-i; b = |a|; then Sin(2pi*a), Sin(-2pi*b + pi/2).
v*b_k folds into the decoder features (bf16 per-partition tensor_scalar).
Encoder padding mask enters PSUM as a -1e30 seed via a K=1 rank-1 bf16 matmul;
softmax runs without max-shift (bf16 exp + f32 accum); the decoder mask folds
into the 1/sum scale. Inputs arrive via 4 packed DMAs; transposes/casts are
host-side layout prep.
"""
import os
import sys

import numpy as np

if "/opt/trn_rl_repo" not in sys.path:
    sys.path.insert(0, "/opt/trn_rl_repo")

S, T, B, H = 512, 256, 8, 128
F = 4
OMEGA = np.array(
    [0.28378837870584145, 0.8460440611349613,
     1.5065032153851483, 2.6337314948021557], dtype=np.float64
)
BK = np.array(
    [1.2424371140304091, 0.31247076876828433,
     0.16201975442700692, 0.045413278208112536], dtype=np.float64
)
TWO_PI = float(2.0 * np.pi)
PI = float(np.pi)
HALF_PI = float(0.5 * np.pi)
NEG_BIG = -1.0e30

_CACHE = {}
LAST_EXEC_NS = None


def _try_install_trace_hook():
    """Best-effort NTFF profile hook for axon (used only when tracing)."""
    try:
        import contextlib
        import ctypes
        import types

        if "antenv.axon_hooks" in sys.modules:
            return
        lib = ctypes.CDLL("/opt/axon/libaxon_pjrt.so")
        if not hasattr(lib, "axon_start_nrt_profile"):
            return
        lib.axon_start_nrt_profile.argtypes = [
            ctypes.POINTER(ctypes.c_int64),
            ctypes.c_size_t,
        ]
        lib.axon_start_nrt_profile.restype = ctypes.c_int64
        lib.axon_stop_nrt_profile.argtypes = [ctypes.c_char_p]
        lib.axon_stop_nrt_profile.restype = ctypes.c_int64

        @contextlib.contextmanager
        def _hook(output_dir, device_ids):
            import jax

            jax.devices()
            if device_ids:
                ids = (ctypes.c_int64 * len(device_ids))(*device_ids)
                rc = lib.axon_start_nrt_profile(ids, len(device_ids))
            else:
                rc = lib.axon_start_nrt_profile(None, 0)
            if rc != 0:
                raise RuntimeError(f"axon_start_nrt_profile rc={rc}")
            try:
                yield
            finally:
                n = lib.axon_stop_nrt_profile(str(output_dir).encode())
                if n < 0:
                    raise RuntimeError(f"axon_stop_nrt_profile rc={n}")

        mod = types.ModuleType("antenv.axon_hooks")
        _h = _hook

        def set_axon_ntff_profile_hook(h):
            pass

        def get_axon_ntff_profile_hook():
            return _h

        mod.set_axon_ntff_profile_hook = set_axon_ntff_profile_hook
        mod.get_axon_ntff_profile_hook = get_axon_ntff_profile_hook
        sys.modules["antenv.axon_hooks"] = mod
        import antenv

        antenv.axon_hooks = mod
    except Exception:
        pass


def _build():
    if "nc" in _CACHE:
        return _CACHE["nc"]
    import concourse.bacc as bacc
    import concourse.tile as tile
    import concourse.mybir as mybir

    F32 = mybir.dt.float32
    BF16 = mybir.dt.bfloat16
    FP16 = mybir.dt.float16
    AF = mybir.ActivationFunctionType
    AL = mybir.AluOpType

    SCAL = [float(w / (2.0 * np.pi)) for w in OMEGA]

    nc = bacc.Bacc("TRN2", target_bir_lowering=False, debug=False, num_devices=8)

    PACKC = (H + T) + (H + S) + 2 * (F + 2)
    pk_d = nc.dram_tensor("pack", [H, PACKC], BF16, kind="ExternalInput")
    em_d = nc.dram_tensor("encmask", [1, S], BF16, kind="ExternalInput")
    out_d = nc.dram_tensor("out", [T, S], F32, kind="ExternalOutput")

    with tile.TileContext(nc) as tc:
        with (
            tc.tile_pool(name="cst", bufs=1) as cst,
            tc.tile_pool(name="wrk", bufs=1) as wrk,
            tc.tile_pool(name="ps", bufs=1, space="PSUM") as psp,
        ):
            # ---- one packed input DMA ([W2|decT|W1|encT|vb:dm-bits]) + em ----
            with nc.named_scope("dma_in"):
                pk_sb = cst.tile([H, PACKC], BF16)
                nc.sync.dma_start(pk_sb[:], pk_d[:])
                em_sb = cst.tile([1, S], BF16)
                nc.sync.dma_start(em_sb[:], em_d[:])
            p1 = pk_sb[:, 0:H + T]
            p2 = pk_sb[:, H + T:2 * H + T + S]
            p3 = pk_sb[:, 2 * H + T + S:PACKC].bitcast(F32)

            ones_sb = cst.tile([1, H], BF16)
            nc.gpsimd.memset(ones_sb[:], 1.0)
            hp_sb = cst.tile([128, 1], F32)
            nc.gpsimd.memset(hp_sb[:], HALF_PI)

            # Preload the Exp table set into the second table slot while
            # ACT is idle, so the softmax needs no mid-stream table load.
            scr = wrk.tile([128, 1], F32, name="scr")
            nc.scalar.activation(scr[:], hp_sb[:], AF.Exp)

            # ---- u = [uE(512) | uD(256)] in PSUM, then copy to SBUF with
            # exactly one reader per PSUM bank (ACT: d-half, DVE: e-half) so
            # downstream consumers never serialize on PSUM bank access ----
            u_ps = psp.tile([128, 768], F32, tag="ups")
            with nc.named_scope("u_mm"):
                nc.tensor.matmul(
                    u_ps[:, 0:S], p2[:, 0:H], p2[:, H:], start=True, stop=True)
                nc.tensor.matmul(
                    u_ps[:, S:], p1[:, 0:H], p1[:, H:], start=True, stop=True)
            u_sb = wrk.tile([128, 768], F32, name="u_sb")
            uA = u_sb[:, 0:S + T]
            with nc.named_scope("u_copy"):
                nc.vector.tensor_scalar_mul(u_sb[:, 0:S], u_ps[:, 0:S], 1.0)
                nc.scalar.copy(u_sb[:, S:], u_ps[:, S:])

            W = S + T  # 768: all feature tiles are [e(512) | d(256)]
            M32 = float(1.5 * 2**23)

            def folds(k, fS, fC):
                """v*b_k folds into d-side features -> new [128,T] bf16 tiles."""
                with nc.named_scope(f"vfold_{k}"):
                    fSdv = wrk.tile([128, T], BF16, name=f"fSdv{k}")
                    nc.vector.tensor_scalar_mul(
                        fSdv[:], fS[:, S:], p3[:, k:k + 1])
                    fCdv = wrk.tile([128, T], BF16, name=f"fCdv{k}")
                    nc.vector.tensor_scalar_mul(
                        fCdv[:], fC[:, S:], p3[:, k:k + 1])
                return fSdv, fCdv

            sc = []

            def scores(k, fSdv, fCdv, fS, fC):
                with nc.named_scope(f"scores_{k}"):
                    last = k == F - 1
                    for tb in range(2):
                        dsl = slice(tb * 128, (tb + 1) * 128)
                        nc.tensor.matmul(
                            sc[tb][:], fSdv[:, dsl], fC[:, 0:S],
                            start=False, stop=False, skip_group_check=True,
                        )
                        nc.tensor.matmul(
                            sc[tb][:], fCdv[:, dsl], fS[:, 0:S],
                            start=False, stop=last, skip_group_check=True,
                        )

            # ---- ACT stream: fS0,fC0,fS1,fC1 directly from u (LUT clamp);
            # DVE/GPSIMD run the k=2,3 range-reduction chains meanwhile ----
            with nc.named_scope("sin_0"):
                fS0 = wrk.tile([128, W], BF16, name="fS0")
                nc.scalar.activation(fS0[:], uA, AF.Sin, scale=float(OMEGA[0]))
                fC0 = wrk.tile([128, W], BF16, name="fC0")
                nc.scalar.activation(
                    fC0[:], uA, AF.Sin, bias=hp_sb[:], scale=float(OMEGA[0]))

            absu = wrk.tile([128, W], F32, name="absu")
            nc.vector.scalar_tensor_tensor(
                absu[:], uA, -1.0, uA, AL.mult, AL.max)

            with nc.named_scope("sin_1"):
                fS1 = wrk.tile([128, W], BF16, name="fS1")
                nc.scalar.activation(fS1[:], uA, AF.Sin, scale=float(OMEGA[1]))
                fC1 = wrk.tile([128, W], BF16, name="fC1")
                nc.scalar.activation(
                    fC1[:], absu[:], AF.Sin, bias=hp_sb[:],
                    scale=float(-OMEGA[1]))

            chain = {}
            for k in (2, 3):
                with nc.named_scope(f"red_{k}"):
                    vv = wrk.tile([128, W], F32, name=f"v{k}")
                    nc.vector.tensor_scalar_mul(vv[:], uA, SCAL[k])
                    ii = wrk.tile([128, W], F32, name=f"i{k}")
                    nc.vector.tensor_scalar(
                        ii[:], vv[:], M32, M32, AL.add, AL.subtract)
                    aa = wrk.tile([128, W], F32, name=f"a{k}")
                    nc.gpsimd.tensor_tensor(aa[:], vv[:], ii[:], AL.subtract)
                    chain[k] = aa

            # score PSUM seeded with -1e30 encoder mask (emitted late so the
            # scheduler keeps it off the critical path)
            for tb in range(2):
                sc_tile = psp.tile([128, S], F32, tag=f"sc{tb}")
                sc.append(sc_tile)
                with nc.named_scope(f"mask_{tb}"):
                    nc.tensor.matmul(
                        sc_tile[:], ones_sb[:], em_sb[:],
                        start=True, stop=False, skip_group_check=True,
                    )

            fS0v, fC0v = folds(0, fS0, fC0)
            scores(0, fS0v, fC0v, fS0, fC0)
            fS1v, fC1v = folds(1, fS1, fC1)
            scores(1, fS1v, fC1v, fS1, fC1)

            for k in (2, 3):
                aa = chain[k]
                with nc.named_scope(f"red2_{k}"):
                    bb = wrk.tile([128, W], F32, name=f"b{k}")
                    nc.vector.scalar_tensor_tensor(
                        bb[:], aa[:], -1.0, aa[:], AL.mult, AL.max)
                with nc.named_scope(f"sin_{k}"):
                    fSk = wrk.tile([128, W], BF16, name=f"fS{k}")
                    nc.scalar.activation(fSk[:], aa[:], AF.Sin, scale=TWO_PI)
                    fCk = wrk.tile([128, W], BF16, name=f"fC{k}")
                    nc.scalar.activation(
                        fCk[:], bb[:], AF.Sin, bias=hp_sb[:], scale=-TWO_PI)
                fSkv, fCkv = folds(k, fSk, fCk)
                scores(k, fSkv, fCkv, fSk, fCk)

            # ---- softmax + decoder-mask scale + store ----
            for tb in range(2):
                with nc.named_scope(f"softmax_{tb}"):
                    ex = wrk.tile([128, S], BF16, name=f"ex{tb}")
                    rs = wrk.tile([128, 1], F32, name=f"rs{tb}")
                    nc.scalar.activation(ex[:], sc[tb][:], AF.Exp, accum_out=rs[:])
                    ri = wrk.tile([128, 1], F32, name=f"ri{tb}")
                    nc.vector.reciprocal(ri[:], rs[:])
                    fac = wrk.tile([128, 1], F32, name=f"fac{tb}")
                    nc.vector.tensor_tensor(
                        fac[:], ri[:], p3[:, F + tb:F + tb + 1],
                        mybir.AluOpType.mult)
                    ot = wrk.tile([128, S], F32, name=f"ot{tb}")
                    nc.vector.tensor_scalar_mul(ot[:], ex[:], fac[:])
                    nc.sync.dma_start(out_d[tb * 128:(tb + 1) * 128, :], ot[:])

    nc.compile()
    _CACHE["nc"] = nc
    return nc


def kernel(encoder_output, decoder_output, W1, W2, v, enc_lens, dec_lens):
    global LAST_EXEC_NS
    from concourse.bass_utils import run_bass_kernel_spmd
    import ml_dtypes

    BF = ml_dtypes.bfloat16
    enc = np.asarray(encoder_output, dtype=np.float32)
    dec = np.asarray(decoder_output, dtype=np.float32)
    W1 = np.asarray(W1, dtype=np.float32)
    W2 = np.asarray(W2, dtype=np.float32)
    v = np.asarray(v, dtype=np.float32)
    enc_lens = np.asarray(enc_lens)
    dec_lens = np.asarray(dec_lens)

    vb = (v[:, None].astype(np.float64) * BK[None, :]).astype(np.float32)  # (H,F)

    in_maps = []
    for b in range(B):
        p1 = np.concatenate([W2, dec[:, b, :].T], axis=1).astype(BF)
        p2 = np.concatenate([W1, enc[:, b, :].T], axis=1).astype(BF)
        dm = (np.arange(T) < int(dec_lens[b])).astype(np.float32)
        p3 = np.ascontiguousarray(
            np.concatenate([vb, dm.reshape(H, 2, order="F")], axis=1),
            dtype=np.float32)
        p3_bits = p3.view(np.uint16).view(BF)  # raw f32 bits as bf16 pairs
        pack = np.ascontiguousarray(np.concatenate([p1, p2, p3_bits], axis=1))
        em = np.where(
            np.arange(S)[None, :] < int(enc_lens[b]), 0.0, NEG_BIG
        ).astype(BF)
        in_maps.append({"pack": pack, "encmask": em})

    trace = os.environ.get("KERNEL_TRACE", "0") == "1"
    if trace:
        _try_install_trace_hook()
    nc = _build()
    ncores = int(os.environ.get("KERNEL_CORES", str(B)))
    res = run_bass_kernel_spmd(nc, in_maps[:ncores], core_ids=list(range(ncores)), trace=trace)
    if trace:
        LAST_EXEC_NS = res.exec_time_ns
        _CACHE["last_res"] = res

    out = np.zeros((T, B, S), dtype=np.float32)
    for b in range(ncores):
        out[:, b, :] = res.results[b]["out"]
    return out
